# revision 8
# baseline (speedup 1.0000x reference)
"""Multi-head attention block (B=32,S=512,D=768,H=12) on 8 TRN2 NeuronCores.

Sharding: data-parallel over batch (4 batches/core), weights replicated,
no collectives. Host pre-transposes x and the weight matrices so the
device kernel is a pure matmul pipeline (no on-chip transposes):

  per core (4 batches):
    yT[o,t]  = (Wqkv xT) for q,k rows   (o on partitions -> per-partition bias)
    v[t,o]   natural (bias via K=1 ones-row matmul), stored with an
             interleaved all-ones column per head: [v_h | 1] is the
             stationary operand of the av matmul, so row 64 of the av
             output is the softmax denominator for free.
    per head: scoresT[s,t] = kT^T qT (K=64), exp on ACT (scale folded),
             av+sums in one matmul, DVE reciprocal + gpsimd
             partition-broadcast + DVE multiply for normalization.
    out[t,:] = avT^T WpT + (proj_b + bv Wp^T)   (bias via K=1 matmul)

All matmuls run as float32r (TF32-like single-pass mode, full PE rate at
N>=256); accumulation is fp32 in PSUM.
"""

import sys

if "/opt/trn_rl_repo" not in sys.path:
    sys.path.insert(0, "/opt/trn_rl_repo")

from contextlib import ExitStack

import numpy as np

import concourse.tile as tile
from concourse import bacc, mybir
from concourse.bass_utils import run_bass_kernel_spmd

B, S, D = 32, 512, 768
H, HD = 12, 64
SCALE = HD**-0.5
NCORES = 8
NB = B // NCORES  # batches per core
P = 128
TCH = S // P  # token chunks per batch
DCH = D // P  # d chunks
QKC = 2 * D // P  # o-chunks holding q,k
NHALF = D // 2  # 384: N-tile for v/proj matmuls
F32 = mybir.dt.float32
F32R = mybir.dt.float32r
EXP = mybir.ActivationFunctionType.Exp


def build_nc():
    nc = bacc.Bacc(None, target_bir_lowering=False, debug=False)
    xT = nc.declare_dram_parameter("xT", [NB, D, S], F32, isOutput=False)
    wqkvT = nc.declare_dram_parameter("wqkvT", [D, 3 * D], F32, isOutput=False)
    wpT = nc.declare_dram_parameter("wpT", [D, D], F32, isOutput=False)
    bqkv = nc.declare_dram_parameter("bqkv", [3 * D], F32, isOutput=False)
    combo = nc.declare_dram_parameter("combo", [D], F32, isOutput=False)
    out = nc.declare_dram_parameter("out", [NB, S, D], F32, isOutput=True)

    with ExitStack() as ctx:
        tc = ctx.enter_context(tile.TileContext(nc))
        wp = ctx.enter_context(tc.tile_pool(name="weights", bufs=1))
        sb = ctx.enter_context(tc.tile_pool(name="work", bufs=1))
        ps = ctx.enter_context(tc.tile_pool(name="psum", bufs=1, space="PSUM"))

        # ---- persistent weights / constants ----
        wq_t = []
        for d in range(DCH):
            t = wp.tile([P, 3 * D], F32R, name=f"wqkvT{d}", tag=f"wqkvT{d}")
            nc.sync.dma_start(out=t, in_=wqkvT[d * P : (d + 1) * P, :].bitcast(F32R))
            wq_t.append(t)
        wp_t = []
        for d in range(DCH):
            t = wp.tile([P, D], F32R, name=f"wpT{d}", tag=f"wpT{d}")
            nc.sync.dma_start(out=t, in_=wpT[d * P : (d + 1) * P, :].bitcast(F32R))
            wp_t.append(t)
        bcols = []
        for c in range(QKC):
            t = wp.tile([P, 1], F32, name=f"bcol{c}", tag=f"bcol{c}")
            nc.sync.dma_start(
                out=t, in_=bqkv[c * P : (c + 1) * P].rearrange("(p o) -> p o", o=1)
            )
            bcols.append(t)
        bvrow = wp.tile([1, D], F32R, name="bvrow", tag="bvrow")
        nc.sync.dma_start(out=bvrow, in_=bqkv[2 * D :].rearrange("(o f) -> o f", o=1).bitcast(F32R))
        comborow = wp.tile([1, D], F32R, name="comborow", tag="comborow")
        nc.sync.dma_start(out=comborow, in_=combo.rearrange("(o f) -> o f", o=1).bitcast(F32R))
        ones_f32 = wp.tile([1, P], F32, name="ones_f32", tag="ones_f32")
        nc.vector.memset(ones_f32, 1.0)
        ones = wp.tile([1, P], F32R, name="ones", tag="ones")
        nc.vector.tensor_copy(ones, ones_f32)
        onescol_f32 = wp.tile([P, H], F32, name="onescol_f32", tag="onescol_f32")
        nc.vector.memset(onescol_f32, 1.0)

        for b in range(NB):
            # ---- load xT for this batch ----
            xt = []
            for d in range(DCH):
                t = sb.tile([P, S], F32R, name=f"xT_b{b}_{d}", tag=f"xT{d}", bufs=1)
                nc.sync.dma_start(out=t, in_=xT[b, d * P : (d + 1) * P, :].bitcast(F32R))
                xt.append(t)

            # ---- q,k in transposed layout: yT[o,t], o-chunks 0..11 ----
            yt = []
            for c in range(QKC):
                pt = ps.tile([P, S], F32, name=f"yTps_b{b}_{c}", tag="mm", bufs=2)
                for d in range(DCH):
                    nc.tensor.matmul(
                        out=pt,
                        lhsT=wq_t[d][:, c * P : (c + 1) * P],
                        rhs=xt[d],
                        start=(d == 0),
                        stop=(d == DCH - 1),
                    )
                st = sb.tile([P, S], F32R, name=f"yT_b{b}_{c}", tag=f"yT{c}", bufs=1)
                nc.vector.tensor_scalar_add(st, pt, bcols[c])
                yt.append(st)

            # ---- v natural with interleaved ones columns: [128, 12*65] ----
            vt = []
            for ti in range(TCH):
                vtile = sb.tile(
                    [P, H * (HD + 1)], F32R, name=f"v_b{b}_{ti}", tag=f"v{ti}", bufs=2
                )
                nc.vector.tensor_copy(
                    vtile.rearrange("p (h k) -> p h k", k=HD + 1)[:, :, HD : HD + 1],
                    onescol_f32.rearrange("p (h o) -> p h o", o=1),
                )
                for half in range(2):
                    pv = ps.tile(
                        [P, NHALF], F32, name=f"vps_b{b}_{ti}_{half}", tag="mm", bufs=2
                    )
                    o0 = 2 * D + half * NHALF
                    nc.tensor.matmul(
                        out=pv,
                        lhsT=ones,
                        rhs=bvrow[:, half * NHALF : (half + 1) * NHALF],
                        start=True,
                        stop=False,
                    )
                    for d in range(DCH):
                        nc.tensor.matmul(
                            out=pv,
                            lhsT=xt[d][:, ti * P : (ti + 1) * P],
                            rhs=wq_t[d][:, o0 : o0 + NHALF],
                            start=False,
                            stop=(d == DCH - 1),
                        )
                    nc.vector.tensor_copy(
                        vtile.rearrange("p (h k) -> p h k", k=HD + 1)[
                            :, 6 * half : 6 * (half + 1), 0:HD
                        ],
                        pv.rearrange("p (h k) -> p h k", k=HD),
                    )
                vt.append(vtile)

            # ---- attention heads ----
            avt = [
                sb.tile([P, S], F32R, name=f"avT_b{b}_{c}", tag=f"avT{c}", bufs=1)
                for c in range(DCH)
            ]
            for h in range(H):
                hp = (h % 2) * HD
                qs = yt[h // 2][hp : hp + HD, :]
                ks = yt[6 + h // 2][hp : hp + HD, :]
                exps = []
                for sc2 in range(2):
                    pt = ps.tile(
                        [P, 2 * S], F32, name=f"sc_b{b}_h{h}_{sc2}", tag="sc", bufs=2
                    )
                    for jj in range(2):
                        j = 2 * sc2 + jj
                        nc.tensor.matmul(
                            out=pt[:, jj * S : (jj + 1) * S],
                            lhsT=ks[:, j * P : (j + 1) * P],
                            rhs=qs,
                            start=True,
                            stop=True,
                        )
                    et = sb.tile(
                        [P, 2 * S], F32R, name=f"expT_b{b}_h{h}_{sc2}", tag="expT",
                        bufs=3,
                    )
                    nc.scalar.activation(et, pt, EXP, scale=SCALE)
                    exps.append(et)
                pav = ps.tile([HD + 1, S], F32, name=f"av_b{b}_h{h}", tag="av", bufs=2)
                for j in range(TCH):
                    nc.tensor.matmul(
                        out=pav,
                        lhsT=vt[j][:, h * (HD + 1) : (h + 1) * (HD + 1)],
                        rhs=exps[j // 2][:, (j % 2) * S : (j % 2 + 1) * S],
                        start=(j == 0),
                        stop=(j == TCH - 1),
                    )
                rt = sb.tile([HD + 1, S], F32, name=f"recip_b{b}_h{h}", tag="recip",
                             bufs=2)
                nc.vector.reciprocal(rt[HD : HD + 1, :], pav[HD : HD + 1, :])
                # partition_broadcast reads physical partition 0, so hop the
                # reciprocal row down from partition 64 via SBUF->SBUF DMA.
                rrow = sb.tile([1, S], F32, name=f"rrow_b{b}_h{h}", tag="rrow",
                               bufs=3)
                nc.sync.dma_start(out=rrow, in_=rt[HD : HD + 1, :])
                bc = sb.tile([HD, S], F32, name=f"bc_b{b}_h{h}", tag="bc", bufs=3)
                nc.gpsimd.partition_broadcast(bc, rrow)
                c = h // 2
                if h % 2 == 0:
                    nc.vector.tensor_mul(avt[c][:HD, :], pav[:HD, :], bc)
                else:
                    tmp = sb.tile([HD, S], F32R, name=f"avtmp_b{b}_h{h}", tag="avtmp",
                                  bufs=2)
                    nc.vector.tensor_mul(tmp, pav[:HD, :], bc)
                    nc.sync.dma_start(out=avt[c][HD : 2 * HD, :], in_=tmp)

            # ---- output projection ----
            for ti in range(TCH):
                ft = sb.tile([P, D], F32, name=f"fin_b{b}_{ti}", tag="fin", bufs=3)
                for half in range(2):
                    pf = ps.tile(
                        [P, NHALF], F32, name=f"fps_b{b}_{ti}_{half}", tag="mm", bufs=2
                    )
                    nc.tensor.matmul(
                        out=pf,
                        lhsT=ones,
                        rhs=comborow[:, half * NHALF : (half + 1) * NHALF],
                        start=True,
                        stop=False,
                    )
                    for d in range(DCH):
                        nc.tensor.matmul(
                            out=pf,
                            lhsT=avt[d][:, ti * P : (ti + 1) * P],
                            rhs=wp_t[d][:, half * NHALF : (half + 1) * NHALF],
                            start=False,
                            stop=(d == DCH - 1),
                        )
                    nc.scalar.copy(ft[:, half * NHALF : (half + 1) * NHALF], pf)
                nc.sync.dma_start(out=out[b, ti * P : (ti + 1) * P, :], in_=ft)

    nc.compile()
    return nc


_CACHE = {}


def _get_nc():
    if "nc" not in _CACHE:
        _CACHE["nc"] = build_nc()
    return _CACHE["nc"]


def _prepare_in_maps(x, qkv_w, qkv_b, proj_w, proj_b):
    x = np.asarray(x, dtype=np.float32)
    qkv_w = np.asarray(qkv_w, dtype=np.float32)
    qkv_b = np.asarray(qkv_b, dtype=np.float32)
    proj_w = np.asarray(proj_w, dtype=np.float32)
    proj_b = np.asarray(proj_b, dtype=np.float32)
    wqkvT = np.ascontiguousarray(qkv_w.T)
    wpT = np.ascontiguousarray(proj_w.T)
    combo = proj_b  # v-bias flows through softmax (rows sum to 1) via bvrow
    in_maps = []
    for c in range(NCORES):
        xs = x[c * NB : (c + 1) * NB]
        xTs = np.ascontiguousarray(xs.transpose(0, 2, 1))
        in_maps.append(
            {
                "xT": xTs,
                "wqkvT": wqkvT,
                "wpT": wpT,
                "bqkv": qkv_b,
                "combo": combo,
            }
        )
    return in_maps


def kernel(x, qkv_w, qkv_b, proj_w, proj_b):
    nc = _get_nc()
    in_maps = _prepare_in_maps(x, qkv_w, qkv_b, proj_w, proj_b)
    res = run_bass_kernel_spmd(nc, in_maps, core_ids=list(range(NCORES)))
    return np.concatenate([res.results[i]["out"] for i in range(NCORES)], axis=0)


# revision 11
# speedup vs baseline: 1.0138x; 1.0138x over previous
"""Multi-head attention block (B=32,S=512,D=768,H=12) on 8 TRN2 NeuronCores.

Sharding: data-parallel over batch (4 batches/core), weights replicated,
no collectives. Host pre-transposes x and the weight matrices so the
device kernel is a pure matmul pipeline (no on-chip transposes):

  per core (4 batches):
    yT[o,t]  = (Wqkv xT) for q,k rows   (o on partitions -> per-partition bias)
    v[t,o]   natural (bias via K=1 ones-row matmul), stored with an
             interleaved all-ones column per head: [v_h | 1] is the
             stationary operand of the av matmul, so row 64 of the av
             output is the softmax denominator for free.
    per head: scoresT[s,t] = kT^T qT (K=64), exp on ACT (scale folded),
             av+sums in one matmul, DVE reciprocal + gpsimd
             partition-broadcast + DVE multiply for normalization.
    out[t,:] = avT^T WpT + (proj_b + bv Wp^T)   (bias via K=1 matmul)

All matmuls run as float32r (TF32-like single-pass mode, full PE rate at
N>=256); accumulation is fp32 in PSUM.
"""

import sys

if "/opt/trn_rl_repo" not in sys.path:
    sys.path.insert(0, "/opt/trn_rl_repo")

from contextlib import ExitStack

import numpy as np

import concourse.tile as tile
from concourse import bacc, mybir
from concourse.bass_utils import run_bass_kernel_spmd

B, S, D = 32, 512, 768
H, HD = 12, 64
SCALE = HD**-0.5
NCORES = 8
NB = B // NCORES  # batches per core
P = 128
TCH = S // P  # token chunks per batch
DCH = D // P  # d chunks
QKC = 2 * D // P  # o-chunks holding q,k
NHALF = D // 2  # 384: N-tile for v/proj matmuls
F32 = mybir.dt.float32
F32R = mybir.dt.float32r
EXP = mybir.ActivationFunctionType.Exp


def _act_reciprocal(nc, out_ap, in_ap):
    """Raw ACT-table reciprocal (~1e-3 rel for |x| >= ~2.5; softmax sums here
    are >= ~50). The bass wrapper refuses Reciprocal for general use; emit
    InstActivation directly."""
    eng = nc.scalar
    ins_ = [eng.lower_ap(in_ap)]
    for arg in (0.0, 1.0, 0.0):  # bias, scale, alpha
        ins_.append(mybir.ImmediateValue(dtype=F32, value=arg))
    return eng.add_instruction(
        mybir.InstActivation(
            name=eng.bass.get_next_instruction_name(),
            func=mybir.ActivationFunctionType.Reciprocal,
            ins=ins_,
            outs=[eng.lower_ap(out_ap)],
        )
    )


def build_nc():
    nc = bacc.Bacc(None, target_bir_lowering=False, debug=False)
    xT = nc.declare_dram_parameter("xT", [NB, D, S], F32, isOutput=False)
    wqkvT = nc.declare_dram_parameter("wqkvT", [D, 3 * D], F32, isOutput=False)
    wpT = nc.declare_dram_parameter("wpT", [D, D], F32, isOutput=False)
    bqkv = nc.declare_dram_parameter("bqkv", [3 * D], F32, isOutput=False)
    combo = nc.declare_dram_parameter("combo", [D], F32, isOutput=False)
    out = nc.declare_dram_parameter("out", [NB, S, D], F32, isOutput=True)

    with ExitStack() as ctx:
        tc = ctx.enter_context(tile.TileContext(nc))
        wp = ctx.enter_context(tc.tile_pool(name="weights", bufs=1))
        sb = ctx.enter_context(tc.tile_pool(name="work", bufs=1))
        ps = ctx.enter_context(tc.tile_pool(name="psum", bufs=1, space="PSUM"))

        # ---- persistent weights / constants ----
        wq_t = []
        for d in range(DCH):
            t = wp.tile([P, 3 * D], F32R, name=f"wqkvT{d}", tag=f"wqkvT{d}")
            nc.sync.dma_start(out=t, in_=wqkvT[d * P : (d + 1) * P, :].bitcast(F32R))
            wq_t.append(t)
        wp_t = []
        for d in range(DCH):
            t = wp.tile([P, D], F32R, name=f"wpT{d}", tag=f"wpT{d}")
            nc.sync.dma_start(out=t, in_=wpT[d * P : (d + 1) * P, :].bitcast(F32R))
            wp_t.append(t)
        bcols = []
        for c in range(QKC):
            t = wp.tile([P, 1], F32, name=f"bcol{c}", tag=f"bcol{c}")
            nc.sync.dma_start(
                out=t, in_=bqkv[c * P : (c + 1) * P].rearrange("(p o) -> p o", o=1)
            )
            bcols.append(t)
        bvrow = wp.tile([1, D], F32R, name="bvrow", tag="bvrow")
        nc.sync.dma_start(out=bvrow, in_=bqkv[2 * D :].rearrange("(o f) -> o f", o=1).bitcast(F32R))
        comborow = wp.tile([1, D], F32R, name="comborow", tag="comborow")
        nc.sync.dma_start(out=comborow, in_=combo.rearrange("(o f) -> o f", o=1).bitcast(F32R))
        ones_f32 = wp.tile([1, P], F32, name="ones_f32", tag="ones_f32")
        nc.vector.memset(ones_f32, 1.0)
        ones = wp.tile([1, P], F32R, name="ones", tag="ones")
        nc.vector.tensor_copy(ones, ones_f32)
        onescol_f32 = wp.tile([P, H], F32, name="onescol_f32", tag="onescol_f32")
        nc.vector.memset(onescol_f32, 1.0)

        for b in range(NB):
            # ---- load xT for this batch ----
            xt = []
            for d in range(DCH):
                t = sb.tile([P, S], F32R, name=f"xT_b{b}_{d}", tag=f"xT{d}", bufs=1)
                nc.sync.dma_start(out=t, in_=xT[b, d * P : (d + 1) * P, :].bitcast(F32R))
                xt.append(t)

            # ---- q,k in transposed layout: yT[o,t], o-chunks 0..11 ----
            yt = []
            for c in range(QKC):
                pt = ps.tile([P, S], F32, name=f"yTps_b{b}_{c}", tag="mm", bufs=2)
                for d in range(DCH):
                    nc.tensor.matmul(
                        out=pt,
                        lhsT=wq_t[d][:, c * P : (c + 1) * P],
                        rhs=xt[d],
                        start=(d == 0),
                        stop=(d == DCH - 1),
                    )
                st = sb.tile([P, S], F32R, name=f"yT_b{b}_{c}", tag=f"yT{c}", bufs=1)
                nc.vector.tensor_scalar_add(st, pt, bcols[c])
                yt.append(st)

            # ---- v natural with interleaved ones columns: [128, 12*65] ----
            vt = []
            for ti in range(TCH):
                vtile = sb.tile(
                    [P, H * (HD + 1)], F32R, name=f"v_b{b}_{ti}", tag=f"v{ti}", bufs=2
                )
                nc.vector.tensor_copy(
                    vtile.rearrange("p (h k) -> p h k", k=HD + 1)[:, :, HD : HD + 1],
                    onescol_f32.rearrange("p (h o) -> p h o", o=1),
                )
                for half in range(2):
                    pv = ps.tile(
                        [P, NHALF], F32, name=f"vps_b{b}_{ti}_{half}", tag="mm", bufs=2
                    )
                    o0 = 2 * D + half * NHALF
                    nc.tensor.matmul(
                        out=pv,
                        lhsT=ones,
                        rhs=bvrow[:, half * NHALF : (half + 1) * NHALF],
                        start=True,
                        stop=False,
                    )
                    for d in range(DCH):
                        nc.tensor.matmul(
                            out=pv,
                            lhsT=xt[d][:, ti * P : (ti + 1) * P],
                            rhs=wq_t[d][:, o0 : o0 + NHALF],
                            start=False,
                            stop=(d == DCH - 1),
                        )
                    nc.vector.tensor_copy(
                        vtile.rearrange("p (h k) -> p h k", k=HD + 1)[
                            :, 6 * half : 6 * (half + 1), 0:HD
                        ],
                        pv.rearrange("p (h k) -> p h k", k=HD),
                    )
                vt.append(vtile)

            # ---- attention heads ----
            avt = [
                sb.tile([P, S], F32R, name=f"avT_b{b}_{c}", tag=f"avT{c}", bufs=1)
                for c in range(DCH)
            ]
            for h in range(H):
                hp = (h % 2) * HD
                qs = yt[h // 2][hp : hp + HD, :]
                ks = yt[6 + h // 2][hp : hp + HD, :]
                exps = []
                for j in range(TCH):
                    pt = ps.tile(
                        [P, S], F32, name=f"sc_b{b}_h{h}_{j}", tag="sc", bufs=4
                    )
                    nc.tensor.matmul(
                        out=pt,
                        lhsT=ks[:, j * P : (j + 1) * P],
                        rhs=qs,
                        start=True,
                        stop=True,
                    )
                    et = sb.tile(
                        [P, S], F32R, name=f"expT_b{b}_h{h}_{j}", tag="expT", bufs=6
                    )
                    nc.scalar.activation(et, pt, EXP, scale=SCALE)
                    exps.append(et)
                pav = ps.tile([HD + 1, S], F32, name=f"av_b{b}_h{h}", tag="av", bufs=2)
                for j in range(TCH):
                    nc.tensor.matmul(
                        out=pav,
                        lhsT=vt[j][:, h * (HD + 1) : (h + 1) * (HD + 1)],
                        rhs=exps[j],
                        start=(j == 0),
                        stop=(j == TCH - 1),
                    )
                rt = sb.tile([HD + 1, S], F32, name=f"recip_b{b}_h{h}", tag="recip",
                             bufs=2)
                _act_reciprocal(nc, rt[HD : HD + 1, :], pav[HD : HD + 1, :])
                # partition_broadcast reads physical partition 0, so hop the
                # reciprocal row down from partition 64 via SBUF->SBUF DMA.
                rrow = sb.tile([1, S], F32, name=f"rrow_b{b}_h{h}", tag="rrow",
                               bufs=3)
                nc.sync.dma_start(out=rrow, in_=rt[HD : HD + 1, :])
                bc = sb.tile([HD, S], F32, name=f"bc_b{b}_h{h}", tag="bc", bufs=3)
                nc.gpsimd.partition_broadcast(bc, rrow)
                c = h // 2
                if h % 2 == 0:
                    nc.vector.tensor_mul(avt[c][:HD, :], pav[:HD, :], bc)
                else:
                    tmp = sb.tile([HD, S], F32R, name=f"avtmp_b{b}_h{h}", tag="avtmp",
                                  bufs=2)
                    nc.vector.tensor_mul(tmp, pav[:HD, :], bc)
                    nc.sync.dma_start(out=avt[c][HD : 2 * HD, :], in_=tmp)

            # ---- output projection ----
            for ti in range(TCH):
                ft = sb.tile([P, D], F32, name=f"fin_b{b}_{ti}", tag="fin", bufs=3)
                for half in range(2):
                    pf = ps.tile(
                        [P, NHALF], F32, name=f"fps_b{b}_{ti}_{half}", tag="mm", bufs=2
                    )
                    nc.tensor.matmul(
                        out=pf,
                        lhsT=ones,
                        rhs=comborow[:, half * NHALF : (half + 1) * NHALF],
                        start=True,
                        stop=False,
                    )
                    for d in range(DCH):
                        nc.tensor.matmul(
                            out=pf,
                            lhsT=avt[d][:, ti * P : (ti + 1) * P],
                            rhs=wp_t[d][:, half * NHALF : (half + 1) * NHALF],
                            start=False,
                            stop=(d == DCH - 1),
                        )
                    nc.scalar.copy(ft[:, half * NHALF : (half + 1) * NHALF], pf)
                nc.sync.dma_start(out=out[b, ti * P : (ti + 1) * P, :], in_=ft)

    nc.compile()
    return nc


_CACHE = {}


def _get_nc():
    if "nc" not in _CACHE:
        _CACHE["nc"] = build_nc()
    return _CACHE["nc"]


def _prepare_in_maps(x, qkv_w, qkv_b, proj_w, proj_b):
    x = np.asarray(x, dtype=np.float32)
    qkv_w = np.asarray(qkv_w, dtype=np.float32)
    qkv_b = np.asarray(qkv_b, dtype=np.float32)
    proj_w = np.asarray(proj_w, dtype=np.float32)
    proj_b = np.asarray(proj_b, dtype=np.float32)
    wqkvT = np.ascontiguousarray(qkv_w.T)
    wpT = np.ascontiguousarray(proj_w.T)
    combo = proj_b  # v-bias flows through softmax (rows sum to 1) via bvrow
    in_maps = []
    for c in range(NCORES):
        xs = x[c * NB : (c + 1) * NB]
        xTs = np.ascontiguousarray(xs.transpose(0, 2, 1))
        in_maps.append(
            {
                "xT": xTs,
                "wqkvT": wqkvT,
                "wpT": wpT,
                "bqkv": qkv_b,
                "combo": combo,
            }
        )
    return in_maps


def kernel(x, qkv_w, qkv_b, proj_w, proj_b):
    nc = _get_nc()
    in_maps = _prepare_in_maps(x, qkv_w, qkv_b, proj_w, proj_b)
    res = run_bass_kernel_spmd(nc, in_maps, core_ids=list(range(NCORES)))
    return np.concatenate([res.results[i]["out"] for i in range(NCORES)], axis=0)


# revision 12
# speedup vs baseline: 1.1074x; 1.0924x over previous
"""Multi-head attention block (B=32,S=512,D=768,H=12) on 8 TRN2 NeuronCores.

Sharding: data-parallel over batch (4 batches/core), weights replicated,
no collectives. Host pre-transposes x and the weight matrices so the
device kernel is a pure matmul pipeline (no on-chip transposes):

  per core (4 batches):
    yT[o,t]  = (Wqkv xT) for q,k rows   (o on partitions -> per-partition bias)
    v[t,o]   natural (bias via K=1 ones-row matmul), stored with an
             interleaved all-ones column per head: [v_h | 1] is the
             stationary operand of the av matmul, so row 64 of the av
             output is the softmax denominator for free.
    per head: scoresT[s,t] = kT^T qT (K=64), exp on ACT (scale folded),
             av+sums in one matmul, DVE reciprocal + gpsimd
             partition-broadcast + DVE multiply for normalization.
    out[t,:] = avT^T WpT + (proj_b + bv Wp^T)   (bias via K=1 matmul)

All matmuls run as float32r (TF32-like single-pass mode, full PE rate at
N>=256); accumulation is fp32 in PSUM.
"""

import sys

if "/opt/trn_rl_repo" not in sys.path:
    sys.path.insert(0, "/opt/trn_rl_repo")

from contextlib import ExitStack

import numpy as np

import concourse.tile as tile
from concourse import bacc, mybir
from concourse.bass_utils import run_bass_kernel_spmd

B, S, D = 32, 512, 768
H, HD = 12, 64
SCALE = HD**-0.5
NCORES = 8
NB = B // NCORES  # batches per core
P = 128
TCH = S // P  # token chunks per batch
DCH = D // P  # d chunks
QKC = 2 * D // P  # o-chunks holding q,k
NHALF = D // 2  # 384: N-tile for v/proj matmuls
F32 = mybir.dt.float32
F32R = mybir.dt.float32r
BF16 = mybir.dt.bfloat16
EXP = mybir.ActivationFunctionType.Exp


def _act_reciprocal(nc, out_ap, in_ap):
    """Raw ACT-table reciprocal (~1e-3 rel for |x| >= ~2.5; softmax sums here
    are >= ~50). The bass wrapper refuses Reciprocal for general use; emit
    InstActivation directly."""
    eng = nc.scalar
    ins_ = [eng.lower_ap(in_ap)]
    for arg in (0.0, 1.0, 0.0):  # bias, scale, alpha
        ins_.append(mybir.ImmediateValue(dtype=F32, value=arg))
    return eng.add_instruction(
        mybir.InstActivation(
            name=eng.bass.get_next_instruction_name(),
            func=mybir.ActivationFunctionType.Reciprocal,
            ins=ins_,
            outs=[eng.lower_ap(out_ap)],
        )
    )


def build_nc():
    nc = bacc.Bacc(None, target_bir_lowering=False, debug=False)
    xT = nc.declare_dram_parameter("xT", [NB, D, S], BF16, isOutput=False)
    wqkvT = nc.declare_dram_parameter("wqkvT", [D, 3 * D], BF16, isOutput=False)
    wpT = nc.declare_dram_parameter("wpT", [D, D], BF16, isOutput=False)
    bqkv = nc.declare_dram_parameter("bqkv", [3 * D], F32, isOutput=False)
    combo = nc.declare_dram_parameter("combo", [D], BF16, isOutput=False)
    bv16 = nc.declare_dram_parameter("bv16", [D], BF16, isOutput=False)
    out = nc.declare_dram_parameter("out", [NB, S, D], F32, isOutput=True)

    with ExitStack() as ctx:
        tc = ctx.enter_context(tile.TileContext(nc))
        wp = ctx.enter_context(tc.tile_pool(name="weights", bufs=1))
        sb = ctx.enter_context(tc.tile_pool(name="work", bufs=1))
        ps = ctx.enter_context(tc.tile_pool(name="psum", bufs=1, space="PSUM"))

        # ---- persistent weights / constants ----
        wq_t = []
        for d in range(DCH):
            t = wp.tile([P, 3 * D], BF16, name=f"wqkvT{d}", tag=f"wqkvT{d}")
            nc.sync.dma_start(out=t, in_=wqkvT[d * P : (d + 1) * P, :])
            wq_t.append(t)
        wp_t = []
        for d in range(DCH):
            t = wp.tile([P, D], BF16, name=f"wpT{d}", tag=f"wpT{d}")
            nc.sync.dma_start(out=t, in_=wpT[d * P : (d + 1) * P, :])
            wp_t.append(t)
        bcols = []
        for c in range(QKC):
            t = wp.tile([P, 1], F32, name=f"bcol{c}", tag=f"bcol{c}")
            nc.sync.dma_start(
                out=t, in_=bqkv[c * P : (c + 1) * P].rearrange("(p o) -> p o", o=1)
            )
            bcols.append(t)
        bvrow = wp.tile([1, D], BF16, name="bvrow", tag="bvrow")
        nc.sync.dma_start(out=bvrow, in_=bv16.rearrange("(o f) -> o f", o=1))
        comborow = wp.tile([1, D], BF16, name="comborow", tag="comborow")
        nc.sync.dma_start(out=comborow, in_=combo.rearrange("(o f) -> o f", o=1))
        ones = wp.tile([1, P], BF16, name="ones", tag="ones")
        nc.vector.memset(ones, 1.0)

        for b in range(NB):
            # ---- load xT for this batch ----
            xt = []
            for d in range(DCH):
                t = sb.tile([P, S], BF16, name=f"xT_b{b}_{d}", tag=f"xT{d}", bufs=1)
                nc.sync.dma_start(out=t, in_=xT[b, d * P : (d + 1) * P, :])
                xt.append(t)

            # ---- q,k in transposed layout: yT[o,t], o-chunks 0..11 ----
            yt = []
            for c in range(QKC):
                pt = ps.tile([P, S], F32, name=f"yTps_b{b}_{c}", tag="mm", bufs=2)
                for d in range(DCH):
                    nc.tensor.matmul(
                        out=pt,
                        lhsT=wq_t[d][:, c * P : (c + 1) * P],
                        rhs=xt[d],
                        start=(d == 0),
                        stop=(d == DCH - 1),
                    )
                st = sb.tile([P, S], BF16, name=f"yT_b{b}_{c}", tag=f"yT{c}", bufs=1)
                nc.vector.tensor_scalar_add(st, pt, bcols[c])
                yt.append(st)

            # ---- v natural with interleaved ones columns: [128, 12*65] ----
            vt = []
            for ti in range(TCH):
                vtile = sb.tile(
                    [P, H * (HD + 1)], BF16, name=f"v_b{b}_{ti}", tag=f"v{ti}", bufs=2
                )
                nc.vector.memset(
                    vtile.rearrange("p (h k) -> p h k", k=HD + 1)[:, :, HD : HD + 1],
                    1.0,
                )
                for half in range(2):
                    pv = ps.tile(
                        [P, NHALF], F32, name=f"vps_b{b}_{ti}_{half}", tag="mm", bufs=2
                    )
                    o0 = 2 * D + half * NHALF
                    nc.tensor.matmul(
                        out=pv,
                        lhsT=ones,
                        rhs=bvrow[:, half * NHALF : (half + 1) * NHALF],
                        start=True,
                        stop=False,
                    )
                    for d in range(DCH):
                        nc.tensor.matmul(
                            out=pv,
                            lhsT=xt[d][:, ti * P : (ti + 1) * P],
                            rhs=wq_t[d][:, o0 : o0 + NHALF],
                            start=False,
                            stop=(d == DCH - 1),
                        )
                    nc.vector.tensor_copy(
                        vtile.rearrange("p (h k) -> p h k", k=HD + 1)[
                            :, 6 * half : 6 * (half + 1), 0:HD
                        ],
                        pv.rearrange("p (h k) -> p h k", k=HD),
                    )
                vt.append(vtile)

            # ---- attention heads ----
            avt = [
                sb.tile([P, S], BF16, name=f"avT_b{b}_{c}", tag=f"avT{c}", bufs=1)
                for c in range(DCH)
            ]
            for h in range(H):
                hp = (h % 2) * HD
                qs = yt[h // 2][hp : hp + HD, :]
                ks = yt[6 + h // 2][hp : hp + HD, :]
                exps = []
                for j in range(TCH):
                    pt = ps.tile(
                        [P, S], F32, name=f"sc_b{b}_h{h}_{j}", tag="sc", bufs=3
                    )
                    nc.tensor.matmul(
                        out=pt,
                        lhsT=ks[:, j * P : (j + 1) * P],
                        rhs=qs,
                        start=True,
                        stop=True,
                    )
                    et = sb.tile(
                        [P, S], BF16, name=f"expT_b{b}_h{h}_{j}", tag="expT", bufs=6
                    )
                    nc.scalar.activation(et, pt, EXP, scale=SCALE)
                    exps.append(et)
                pav = ps.tile([HD + 1, S], F32, name=f"av_b{b}_h{h}", tag="av", bufs=3)
                for j in range(TCH):
                    nc.tensor.matmul(
                        out=pav,
                        lhsT=vt[j][:, h * (HD + 1) : (h + 1) * (HD + 1)],
                        rhs=exps[j],
                        start=(j == 0),
                        stop=(j == TCH - 1),
                    )
                rt = sb.tile([HD + 1, S], F32, name=f"recip_b{b}_h{h}", tag="recip",
                             bufs=3)
                _act_reciprocal(nc, rt[HD : HD + 1, :], pav[HD : HD + 1, :])
                # partition_broadcast reads physical partition 0, so hop the
                # reciprocal row down from partition 64 via SBUF->SBUF DMA.
                rrow = sb.tile([1, S], F32, name=f"rrow_b{b}_h{h}", tag="rrow",
                               bufs=3)
                nc.sync.dma_start(out=rrow, in_=rt[HD : HD + 1, :])
                bc = sb.tile([HD, S], F32, name=f"bc_b{b}_h{h}", tag="bc", bufs=3)
                nc.gpsimd.partition_broadcast(bc, rrow)
                c = h // 2
                if h % 2 == 0:
                    nc.vector.tensor_mul(avt[c][:HD, :], pav[:HD, :], bc)
                else:
                    tmp = sb.tile([HD, S], BF16, name=f"avtmp_b{b}_h{h}", tag="avtmp",
                                  bufs=3)
                    nc.vector.tensor_mul(tmp, pav[:HD, :], bc)
                    nc.sync.dma_start(out=avt[c][HD : 2 * HD, :], in_=tmp)

            # ---- output projection ----
            for ti in range(TCH):
                ft = sb.tile([P, D], F32, name=f"fin_b{b}_{ti}", tag="fin", bufs=3)
                for half in range(2):
                    pf = ps.tile(
                        [P, NHALF], F32, name=f"fps_b{b}_{ti}_{half}", tag="mm", bufs=2
                    )
                    nc.tensor.matmul(
                        out=pf,
                        lhsT=ones,
                        rhs=comborow[:, half * NHALF : (half + 1) * NHALF],
                        start=True,
                        stop=False,
                    )
                    for d in range(DCH):
                        nc.tensor.matmul(
                            out=pf,
                            lhsT=avt[d][:, ti * P : (ti + 1) * P],
                            rhs=wp_t[d][:, half * NHALF : (half + 1) * NHALF],
                            start=False,
                            stop=(d == DCH - 1),
                        )
                    nc.scalar.copy(ft[:, half * NHALF : (half + 1) * NHALF], pf)
                nc.sync.dma_start(out=out[b, ti * P : (ti + 1) * P, :], in_=ft)

    nc.compile()
    return nc


_CACHE = {}


def _get_nc():
    if "nc" not in _CACHE:
        _CACHE["nc"] = build_nc()
    return _CACHE["nc"]


def _prepare_in_maps(x, qkv_w, qkv_b, proj_w, proj_b):
    x = np.asarray(x, dtype=np.float32)
    qkv_w = np.asarray(qkv_w, dtype=np.float32)
    qkv_b = np.asarray(qkv_b, dtype=np.float32)
    proj_w = np.asarray(proj_w, dtype=np.float32)
    proj_b = np.asarray(proj_b, dtype=np.float32)
    import ml_dtypes

    bf16 = ml_dtypes.bfloat16
    wqkvT = np.ascontiguousarray(qkv_w.T).astype(bf16)
    wpT = np.ascontiguousarray(proj_w.T).astype(bf16)
    combo = proj_b.astype(bf16)  # v-bias flows through softmax via bvrow
    bv16 = qkv_b[2 * D :].astype(bf16)
    in_maps = []
    for c in range(NCORES):
        xs = x[c * NB : (c + 1) * NB]
        xTs = np.ascontiguousarray(xs.transpose(0, 2, 1)).astype(bf16)
        in_maps.append(
            {
                "xT": xTs,
                "wqkvT": wqkvT,
                "wpT": wpT,
                "bqkv": qkv_b,
                "combo": combo,
                "bv16": bv16,
            }
        )
    return in_maps


def kernel(x, qkv_w, qkv_b, proj_w, proj_b):
    nc = _get_nc()
    in_maps = _prepare_in_maps(x, qkv_w, qkv_b, proj_w, proj_b)
    res = run_bass_kernel_spmd(nc, in_maps, core_ids=list(range(NCORES)))
    return np.concatenate([res.results[i]["out"] for i in range(NCORES)], axis=0)
